# revision 2
# baseline (speedup 1.0000x reference)
"""NLBlockND multi-cross attention block on 8 Trainium2 NeuronCores, v2.

Per-core shard: core c handles batch b = c//2, spatial half h = c%2.
Design vs v1: fp16 input loads (halved DMA, gz projection runs straight off
the fp16 x tiles — no separate bf16 copy); i-chunks of 512 (halved matmul
count, matmuls at the 512-row PSUM-bank max); exp values stored bf16 in
j-major layout (contiguous matmul rhs + contiguous DVE adds at the 2x
16-bit rate); softmax j-reduction as an incremental in-place bf16 add-tree
on DVE (replaces the gpsimd tree + DVE reduce, which together cost more
than the matmuls); exp split across ACT (exact, AF.Exp) and Pool
(Schraudolph bit-trick exp: int16(A*s+B) bitcast as bf16, one tensor_scalar
per group) so the Activation engine stops pacing the main loop; softmax
shift -5 instead of -50 keeps the Schraudolph int16 range positive (shift
cancels in the ratio; fp32/bf16 absorb exp(+-60) fine).  rstd via
exp(-0.5*ln(var+eps)) keeps ACT on the one table with Exp+Ln.  Conv biases
b_g/b_z drop out in training-mode BN; w_z folded into w_g on the host.
BN batch stats all-reduced ([128,2]) across the 8 cores.
"""
import sys
sys.path.insert(0, '/opt/trn_rl_repo')

import numpy as np

B, CIN, CI, H, W = 4, 256, 128, 64, 64
NJ = H * W              # 4096 (full spatial, j axis)
NI = NJ // 2            # 2048 per-core i positions
IC = 512                # i-chunk
NCH = NI // IC          # 4 chunks
JBS = NJ // 128         # 32 j-blocks
NG = JBS // 2           # 16 exp groups (2 j-blocks) per chunk
SHIFT = -5.0
BN_EPS = 1e-5
NTOT = float(B * NJ)    # BN count per channel
LN2 = 0.6931471805599453
A16 = 128.0 / LN2                        # schraudolph bf16 scale
B16 = 127.0 * 128.0 - 5.6 + SHIFT * A16  # bias, incl. softmax shift
# exp groups computed on DVE via schraudolph (rest on ACT, exact); the
# GPSIMD/Pool engine cannot touch PSUM, so it gets SBUF-only tree adds
DVE_G = {0: (6, 14), 1: (2, 6, 10, 14), 2: (2, 6, 10, 14), 3: (2, 6, 10, 14)}
POOL_L1 = (0, 2, 4, 6)   # L1 tree adds done on Pool (SBUF only)

_CACHE = {}


def _build(repeat=1, hoist=True, unroll=1):
    import concourse.bacc as bacc
    import concourse.mybir as mybir
    from concourse import tile

    dt = mybir.dt
    AF = mybir.ActivationFunctionType
    ALU = mybir.AluOpType

    nc = bacc.Bacc("TRN2", target_bir_lowering=False, debug=False, num_devices=8)

    xt = nc.dram_tensor("xt", [CIN, NJ], dt.float16, kind="ExternalInput").ap()
    xo = nc.dram_tensor("xo", [CIN, NI], dt.float16, kind="ExternalInput").ap()
    wtT = nc.dram_tensor("wtT", [CIN, CI], dt.float16, kind="ExternalInput").ap()
    wpT = nc.dram_tensor("wpT", [CIN, CI], dt.float16, kind="ExternalInput").ap()
    wzgT = nc.dram_tensor("wzgT", [CIN, CI], dt.float16, kind="ExternalInput").ap()
    # consts columns: 0 b_theta, 1 b_phi, 2 gamma, 3 beta, 4 SHIFT, 5 eps, 6 1/NTOT
    consts = nc.dram_tensor("consts", [CI, 8], dt.float32, kind="ExternalInput").ap()
    zout_d = nc.dram_tensor("z", [CI, NI], dt.float32, kind="ExternalOutput").ap()

    cc_in = nc.dram_tensor("cc_in", [CI, 2], dt.float32)
    cc_out = nc.dram_tensor("cc_out", [8 * CI, 2], dt.float32, addr_space="Shared")

    with tile.TileContext(nc) as tc:
        with tc.tile_pool(name="big", bufs=1) as bigp, \
             tc.tile_pool(name="exp", bufs=2) as expp, \
             tc.tile_pool(name="tr", bufs=2) as trp, \
             tc.tile_pool(name="S", bufs=3, space="PSUM") as Sp, \
             tc.tile_pool(name="zp", bufs=2, space="PSUM") as zpp, \
             tc.tile_pool(name="ld", bufs=1) as ldp:

          for _rep in range(unroll):
              # ---- DRAM loads, ordered for earliest compute start ----
              wtT_r = [ldp.tile([128, CI], dt.float16, tag=f"wt{c}", name=f"wtT_r{c}") for c in range(2)]
              wpT_r = [ldp.tile([128, CI], dt.float16, tag=f"wp{c}", name=f"wpT_r{c}") for c in range(2)]
              wzgT_r = [ldp.tile([128, CI], dt.float16, tag=f"wz{c}", name=f"wzgT_r{c}") for c in range(2)]
              xo_r = [ldp.tile([128, NI], dt.float16, tag=f"xo{c}", name=f"xo_r{c}") for c in range(2)]
              xt_r = [ldp.tile([128, NJ], dt.float16, tag=f"xt{c}", name=f"xt_r{c}") for c in range(2)]
              cst = bigp.tile([CI, 8], dt.float32, tag="cst")
              nc.sync.dma_start(cst[:], consts[:])
              for c in range(2):
                  nc.sync.dma_start(wpT_r[c][:], wpT[c * 128:(c + 1) * 128, :])
                  nc.sync.dma_start(xo_r[c][:, 0:512], xo[c * 128:(c + 1) * 128, 0:512])
              for c in range(2):
                  nc.sync.dma_start(wtT_r[c][:], wtT[c * 128:(c + 1) * 128, :])
                  nc.sync.dma_start(xt_r[c][:, 0:512], xt[c * 128:(c + 1) * 128, 0:512])
              for c in range(2):
                  nc.sync.dma_start(wzgT_r[c][:], wzgT[c * 128:(c + 1) * 128, :])
              ones_b = bigp.tile([128, 128], dt.bfloat16, tag="ones")
              nc.gpsimd.memset(ones_b[:], 1.0)
              for c in range(2):
                  nc.sync.dma_start(xt_r[c][:, 512:1024], xt[c * 128:(c + 1) * 128, 512:1024])
              for c in range(2):
                  nc.sync.dma_start(xo_r[c][:, 512:NI], xo[c * 128:(c + 1) * 128, 512:NI])
              for p in range(2, 8):
                  cs = slice(p * 512, (p + 1) * 512)
                  for c in range(2):
                      nc.sync.dma_start(xt_r[c][:, cs], xt[c * 128:(c + 1) * 128, cs])

              theta = bigp.tile([128, NJ], dt.float32r, tag="theta")
              gzT = bigp.tile([128, NJ], dt.bfloat16, tag="gzT")
              z_sb = bigp.tile([128, NI], dt.float32, tag="z_sb")
              stat = bigp.tile([128, 2], dt.float32, tag="stat")

              def theta_piece(p):
                  ps = Sp.tile([128, 1024], dt.float32, tag="S", name=f"S_th{p}")
                  cs = slice(p * 512, (p + 1) * 512)
                  for c in range(2):
                      nc.tensor.matmul(ps[:, 0:512], wtT_r[c][:], xt_r[c][:, cs],
                                       start=(c == 0), stop=(c == 1))
                  nc.vector.tensor_scalar_add(theta[:, cs], ps[:, 0:512], cst[:, 0:1])

              def gz_quad(q):
                  ps = Sp.tile([128, 1024], dt.float32, tag="S", name=f"gz{q}")
                  for t in range(4):
                      jb = 4 * q + t
                      for c in range(2):
                          nc.tensor.matmul(ps[:, t * 128:(t + 1) * 128],
                                           xt_r[c][:, jb * 128:(jb + 1) * 128],
                                           wzgT_r[c][:], start=(c == 0), stop=(c == 1))
                  nc.scalar.activation(gzT[:, q * 512:(q + 1) * 512], ps[:, 0:512],
                                       AF.Identity)

              # phi for all chunks, computed in the prologue
              phi_sb = bigp.tile([128, NI], dt.float32r, tag="phi")

              def phi_piece(k):
                  ks = slice(k * IC, (k + 1) * IC)
                  ps = Sp.tile([128, 1024], dt.float32, tag="S", name=f"S_ph{k}")
                  for c in range(2):
                      nc.tensor.matmul(ps[:, 0:512], wpT_r[c][:], xo_r[c][:, ks],
                                       start=(c == 0), stop=(c == 1))
                  nc.vector.tensor_scalar_add(phi_sb[:, ks], ps[:, 0:512],
                                              cst[:, 1:2])

              def lp_add(out, a, b, pool=False):
                  eng = nc.gpsimd if pool else nc.vector
                  with nc.allow_low_precision(reason="bf16 softmax sums"):
                      eng.tensor_add(out, a, b)

              # ---- main chunk loop; chunk k's finishers run inside k+1 ----
              prev = None

              def finish_zparts(st):
                  ek, zpart = st["ek"], st["zpart"]
                  for jb in (JBS - 4, JBS - 3, JBS - 2, JBS - 1):
                      nc.tensor.matmul(zpart[:], gzT[:, jb * 128:(jb + 1) * 128],
                                       ek[:, jb * 512:(jb + 1) * 512],
                                       start=(jb == 0), stop=(jb == JBS - 1))

              def finish_chunk(st):
                  k, zpart, L1 = st["k"], st["zpart"], st["L1"]
                  ks = slice(k * IC, (k + 1) * IC)
                  s_part = trp.tile([128, IC], dt.bfloat16, tag="sp", name=f"sp{k}")
                  lp_add(s_part[:], L1[0][:, 0:512], L1[0][:, 512:1024])
                  rs_t = Sp.tile([128, 1024], dt.float32, tag="S", name=f"rs{k}")
                  rs = rs_t[:, 0:512]
                  nc.tensor.matmul(rs, ones_b[:], s_part[:], start=True, stop=True)
                  rrs = trp.tile([128, IC], dt.float32, tag="rrs", name=f"rrs{k}")
                  nc.vector.reciprocal(rrs[:], rs)
                  nc.vector.tensor_mul(z_sb[:, ks], zpart[:], rrs[:])
                  # per-chunk BN stat partials
                  sq = trp.tile([128, IC], dt.float32, tag="sq", bufs=1,
                                name=f"sq{k}")
                  nc.vector.tensor_mul(sq[:], z_sb[:, ks], z_sb[:, ks])
                  s2c = trp.tile([128, 1], dt.float32, tag="s2c", name=f"s2c{k}")
                  nc.vector.reduce_sum(s2c[:], sq[:], axis=mybir.AxisListType.X)
                  s1c = trp.tile([128, 1], dt.float32, tag="s1c", name=f"s1c{k}")
                  nc.vector.reduce_sum(s1c[:], z_sb[:, ks],
                                       axis=mybir.AxisListType.X)
                  if k == 0:
                      nc.vector.tensor_copy(stat[:, 0:1], s1c[:])
                      nc.vector.tensor_copy(stat[:, 1:2], s2c[:])
                  else:
                      nc.vector.tensor_add(stat[:, 0:1], stat[:, 0:1], s1c[:])
                      nc.vector.tensor_add(stat[:, 1:2], stat[:, 1:2], s2c[:])

              for k in range(NCH):
                  if k == 0:
                      phi_piece(0)
                  ek = expp.tile([128, IC * JBS], dt.bfloat16, tag="e", name=f"e{k}")
                  ei16 = ek[:].bitcast(dt.int16)
                  L1 = [trp.tile([128, 1024], dt.bfloat16, tag=f"L1_{m}",
                                 name=f"L1_{k}_{m}") for m in range(8)]
                  zpart = zpp.tile([128, IC], dt.float32, tag="zp", name=f"zp{k}")
                  st = {"k": k, "ek": ek, "zpart": zpart, "L1": L1}

                  def zpart_mms(g):
                      for jb in (2 * g, 2 * g + 1):
                          nc.tensor.matmul(zpart[:], gzT[:, jb * 128:(jb + 1) * 128],
                                           ek[:, jb * 512:(jb + 1) * 512],
                                           start=(jb == 0), stop=(jb == JBS - 1))

                  for g in range(NG):
                      if k == 0 and g % 2 == 0:
                          theta_piece(g // 2)
                          gz_quad(g // 2)
                          if g in (2, 6, 10):
                              phi_piece(g // 4 + 1)
                      S = Sp.tile([128, 1024], dt.float32, tag="S", name=f"S{k}_{g}")
                      for t in range(2):
                          jb = 2 * g + t
                          nc.tensor.matmul(S[:, t * 512:(t + 1) * 512],
                                           theta[:, jb * 128:(jb + 1) * 128],
                                           phi_sb[:, k * IC:(k + 1) * IC],
                                           start=True, stop=True)
                      gs = slice(g * 1024, (g + 1) * 1024)
                      if g in DVE_G[k]:
                          with nc.allow_low_precision(reason="schraudolph exp"):
                              nc.vector.tensor_scalar(ei16[:, gs], S[:], A16, B16,
                                                      op0=ALU.mult, op1=ALU.add)
                      else:
                          nc.scalar.activation(ek[:, gs], S[:], AF.Exp,
                                               bias=cst[:, 4:5])
                      if g == 1 and prev is not None:
                          finish_zparts(prev)
                      elif g == 4 and prev is not None:
                          finish_chunk(prev)
                      if g > 1:
                          zpart_mms(g - 2)
                      if g % 2 == 1:
                          m = g // 2
                          lp_add(L1[m][:], ek[:, (2 * m) * 1024:(2 * m + 1) * 1024],
                                 ek[:, (2 * m + 1) * 1024:(2 * m + 2) * 1024],
                                 pool=(m in POOL_L1))
                          if m == 1:
                              lp_add(L1[0][:], L1[0][:], L1[1][:])
                          elif m == 3:
                              lp_add(L1[2][:], L1[2][:], L1[3][:])
                              lp_add(L1[0][:], L1[0][:], L1[2][:])
                          elif m == 5:
                              lp_add(L1[4][:], L1[4][:], L1[5][:])
                          elif m == 7:
                              lp_add(L1[6][:], L1[6][:], L1[7][:])
                              lp_add(L1[4][:], L1[4][:], L1[6][:])
                              lp_add(L1[0][:], L1[0][:], L1[4][:])
                  prev = st
              finish_zparts(prev)
              finish_chunk(prev)

              # ---- BN stats AllGather + local reduce ----
              nc.sync.dma_start(cc_in.ap()[:], stat[:])
              nc.gpsimd.collective_compute(
                  "AllGather", mybir.AluOpType.bypass,
                  replica_groups=[list(range(8))],
                  ins=[cc_in.ap().opt()], outs=[cc_out.ap().opt()])
              stat_ag = bigp.tile([128, 16], dt.float32, tag="stat_ag")
              ag_view = cc_out.ap()[:].rearrange("(s p) c -> p s c", s=8)
              nc.gpsimd.dma_start(stat_ag[:].rearrange("p (s c) -> p s c", c=2), ag_view)
              agv = stat_ag[:].rearrange("p (s c) -> p s c", c=2)
              ag4 = bigp.tile([128, 8], dt.float32, tag="ag4")
              ag4v = ag4[:].rearrange("p (s c) -> p s c", c=2)
              nc.vector.tensor_add(ag4v[:], agv[:, 0:4, :], agv[:, 4:8, :])
              ag2 = bigp.tile([128, 4], dt.float32, tag="ag2")
              ag2v = ag2[:].rearrange("p (s c) -> p s c", c=2)
              nc.vector.tensor_add(ag2v[:], ag4v[:, 0:2, :], ag4v[:, 2:4, :])
              stat_all = bigp.tile([128, 2], dt.float32, tag="stat_all")
              nc.vector.tensor_add(stat_all[:], ag2v[:, 0, :], ag2v[:, 1, :])

              # mean = S1/NTOT ; ex2 = S2/NTOT ; var = ex2 - mean^2
              me = bigp.tile([128, 2], dt.float32, tag="me")
              nc.vector.tensor_scalar_mul(me[:], stat_all[:], cst[:, 6:7])
              mean = me[:, 0:1]
              msq = bigp.tile([128, 1], dt.float32, tag="msq")
              nc.vector.tensor_mul(msq[:], mean, mean)
              var = bigp.tile([128, 1], dt.float32, tag="var")
              nc.vector.tensor_sub(var[:], me[:, 1:2], msq[:])
              lnv = bigp.tile([128, 1], dt.float32, tag="lnv")
              nc.scalar.activation(lnv[:], var[:], AF.Ln, bias=cst[:, 5:6])
              rstd = bigp.tile([128, 1], dt.float32, tag="rstd")
              nc.scalar.activation(rstd[:], lnv[:], AF.Exp, scale=-0.5)
              scale = bigp.tile([128, 1], dt.float32, tag="scale")
              nc.vector.tensor_mul(scale[:], rstd[:], cst[:, 2:3])
              mscale = bigp.tile([128, 1], dt.float32, tag="mscale")
              nc.vector.tensor_mul(mscale[:], mean, scale[:])
              bias2 = bigp.tile([128, 1], dt.float32, tag="bias2")
              nc.vector.tensor_sub(bias2[:], cst[:, 3:4], mscale[:])

              # apply + store, split for ACT/DMA overlap
              zfin = bigp.tile([128, NI], dt.float32, tag="zfin")
              for hq in range(4):
                  cs = slice(hq * (NI // 4), (hq + 1) * (NI // 4))
                  nc.scalar.activation(zfin[:, cs], z_sb[:, cs], AF.Identity,
                                       bias=bias2[:], scale=scale[:])
                  nc.gpsimd.dma_start(zout_d[:, cs], zfin[:, cs])

    nc.compile()

    return nc


def _prep_in_maps(inputs):
    xt_full = inputs['x_thisBranch'].reshape(B, CIN, NJ).astype(np.float16)
    xo_full = inputs['x_otherBranch'].reshape(B, CIN, NJ).astype(np.float16)
    wtT = np.ascontiguousarray(inputs['w_theta'].T.astype(np.float16))
    wpT = np.ascontiguousarray(inputs['w_phi'].T.astype(np.float16))
    w_zg = (inputs['w_z'].astype(np.float64) @ inputs['w_g'].astype(np.float64))
    wzgT = np.ascontiguousarray(w_zg.T.astype(np.float16))
    consts = np.zeros((CI, 8), np.float32)
    consts[:, 0] = inputs['b_theta']
    consts[:, 1] = inputs['b_phi']
    consts[:, 2] = inputs['bn_gamma']
    consts[:, 3] = inputs['bn_beta']
    consts[:, 4] = SHIFT
    consts[:, 5] = BN_EPS
    consts[:, 6] = 1.0 / NTOT
    in_maps = []
    for c in range(8):
        b, h = c // 2, c % 2
        in_maps.append({
            "xt": np.ascontiguousarray(xt_full[b]),
            "xo": np.ascontiguousarray(xo_full[b][:, h * NI:(h + 1) * NI]),
            "wtT": wtT, "wpT": wpT, "wzgT": wzgT, "consts": consts,
        })
    return in_maps


def kernel(**inputs):
    from concourse.bass_utils import run_bass_kernel_spmd
    if "nc" not in _CACHE:
        _CACHE["nc"] = _build()
    nc = _CACHE["nc"]
    in_maps = _prep_in_maps(inputs)
    res = run_bass_kernel_spmd(nc, in_maps, list(range(8)))
    out = np.empty((B, CI, NJ), np.float32)
    for c in range(8):
        b, h = c // 2, c % 2
        out[b][:, h * NI:(h + 1) * NI] = res.results[c]["z"]
    return out.reshape(B, CI, H, W)


if __name__ == "__main__":
    inputs = np.load('/tmp/ref_inputs.npy', allow_pickle=True).item()
    ref = np.load('/tmp/ref_output.npy')
    got = kernel(**inputs)
    err = np.abs(got - ref)
    denom = np.abs(ref).max()
    print(f"abs max err: {err.max():.4e}  (ref absmax {denom:.3f})")
    print(f"Relative error: {err.max() / denom:.4e}")
